# revision 48
# baseline (speedup 1.0000x reference)
"""Neural-stack kernel for Trainium2 (8 NeuronCores, SPMD, W-sharded).

Math: the reference recurrence
    stack_t = (noop+pop)_t * stack_{t-1} + push_t * shift_down(stack_{t-1}, v_t)
is linear in the stack given per-step scalars.  All step matrices are
polynomials in the down-shift Z (nilpotent, Z^64 = 0), so products over a
chunk of B steps stay lower-triangular-Toeplitz.  Per chunk c (B=128 steps)
the outputs and the chunk-boundary state obey
    Out_c   = A_c @ S_c + C_c @ V_c          (B x W)
    S_{c+1} = P_c @ S_c + K_c @ V_c          (D x W)
with small coefficient matrices A/C/P/K derived from the (tiny, strictly
sequential, nonlinear) strength recurrence.  The host computes the strength
recurrence + coefficients from `actions` (48KB of input); the 8 cores each
run the chunked matmul recurrence over their 64-column shard of `values`
(99.7% of the FLOPs and bytes).
"""

import sys
import numpy as np

for _p in ("/opt/trn_rl_repo", "/root/.axon_site/_ro/trn_rl_repo"):
    if _p not in sys.path:
        sys.path.append(_p)

T, D, W = 4096, 64, 512
B = 128               # chunk length (time steps per chunk)
NCK = T // B          # 32 chunks
NCORES = 8
WS = W // NCORES      # 64 columns per core
EPS = np.float32(1e-6)


# ---------------------------------------------------------------- host side
def _build_coeffs(actions):
    from numpy.lib.stride_tricks import sliding_window_view

    err = np.seterr(all="ignore")
    a = np.asarray(actions, np.float32)

    # strength recurrence, step-exact f32 (matches the reference's fp32 path)
    s = np.zeros(D, np.float32)
    sig = np.zeros((T + 1, D), np.float32)
    g = np.zeros(T + 1, np.float32)
    for t in range(1, T + 1):
        push, pop, noop = a[t - 1]
        sig[t] = s
        tot = s.sum(dtype=np.float32)
        g[t] = (np.float32(1) / (tot + EPS)) if tot > EPS else np.float32(0)
        cum_above = (np.cumsum(s[::-1], dtype=np.float32)[::-1] - s).astype(np.float32)
        red = np.minimum(np.maximum(pop - cum_above, np.float32(0)), s)
        s = noop * s + pop * (s - red) + push * np.concatenate([[push], s[:-1]]).astype(np.float32)
        tot2 = s.sum(dtype=np.float32)
        if tot2 > D:
            s *= np.float32(D) / tot2

    sv = (a[:, 2] + a[:, 1]).reshape(NCK, B)
    pv = a[:, 0].reshape(NCK, B).copy()
    sigc = sig[1:].reshape(NCK, B, D)
    gc = g[1:].reshape(NCK, B)

    # prefix coefficient vectors rho[c, i] of prod_{tau<=i} (s + p Z) mod Z^D
    rho = np.zeros((NCK, B + 1, D), np.float32)
    rho[:, 0, 0] = 1
    for i in range(1, B + 1):
        st = sv[:, i - 1 : i]
        pt = pv[:, i - 1 : i]
        r = rho[:, i - 1]
        rho[:, i, 0] = st[:, 0] * r[:, 0]
        rho[:, i, 1:] = st * r[:, 1:] + pt * r[:, :-1]

    # A[c, i, k] = g * sum_m sig[c,i,k+m] * rho[c,i,m]
    sig_pad = np.concatenate([sigc, np.zeros((NCK, B, D), np.float32)], axis=2)
    win = sliding_window_view(sig_pad, D, axis=2)
    A = np.einsum("cikm,cim->cik", win[:, :, :D, :].astype(np.float32),
                  rho[:, :B], optimize=True).astype(np.float32)
    A *= gc[:, :, None]

    # interval products q^{(j)} = coeffs of prod_{tau=j+1}^{i-1} (s+pZ) mod Z^D
    # (one slot per start j, evolved in lockstep) -> C and K
    q = np.zeros((NCK, B, D), np.float32)
    q[:, :, 0] = 1.0
    C = np.zeros((NCK, B, B), np.float32)
    for i in range(2, B + 1):
        if i >= 3:
            st = sv[:, i - 2 : i - 1]
            pt = pv[:, i - 2 : i - 1]
            qs = q[:, : i - 2]
            q[:, : i - 2, 1:] = st[:, :, None] * qs[:, :, 1:] + pt[:, :, None] * qs[:, :, :-1]
            q[:, : i - 2, 0] = st * qs[:, :, 0]
        dots = np.einsum("cm,cjm->cj", sigc[:, i - 1], q[:, : i - 1],
                         optimize=True).astype(np.float32)
        C[:, i - 1, : i - 1] = gc[:, i - 1 : i] * pv[:, : i - 1] * dots
    # bring every q^{(j)} up to the full-chunk product prod_{j+1}^{B}
    st = sv[:, B - 1 : B]
    pt = pv[:, B - 1 : B]
    qs = q[:, : B - 1]
    q[:, : B - 1, 1:] = st[:, :, None] * qs[:, :, 1:] + pt[:, :, None] * qs[:, :, :-1]
    q[:, : B - 1, 0] = st * qs[:, :, 0]
    K = (pv[:, None, :] * q.transpose(0, 2, 1)).astype(np.float32)   # [c, D, B]

    P = np.zeros((NCK, D, D), np.float32)
    rB = rho[:, B]
    for k in range(D):
        P[:, k:, k] = rB[:, : D - k]

    np.seterr(**err)
    # device layouts: chunk-stacked transposed (lhsT) matrices
    atp = np.ascontiguousarray(A.transpose(2, 0, 1).reshape(D, NCK * B))    # [64, 4096]
    ctp = np.ascontiguousarray(C.transpose(2, 0, 1).reshape(B, NCK * B))    # [128, 4096]
    ptp = np.ascontiguousarray(P.transpose(2, 0, 1).reshape(D, NCK * D))    # [64, 2048]
    ktp = np.ascontiguousarray(K.transpose(2, 0, 1).reshape(B, NCK * D))    # [128, 2048]
    return atp, ctp, ptp, ktp


# ---------------------------------------------------------------- device side
_PROG = None

# tuning knobs (sweepable): input DMA groups, output DMA groups, PSUM depth
GIN = 4
GOUT = 8
NPS = 2

# input region column offsets inside the [128, INP_W] device tensor
_ATP_W, _CTP_W, _PTP_W, _KTP_W, _VAL_W = NCK * B, NCK * B, NCK * D, NCK * D, NCK * WS
_OFF_ATP = 0
_OFF_CTP = _OFF_ATP + _ATP_W
_OFF_PTP = _OFF_CTP + _CTP_W
_OFF_KTP = _OFF_PTP + _PTP_W
_OFF_VAL = _OFF_KTP + _KTP_W
INP_W = _OFF_VAL + _VAL_W


def _build_program():
    """Raw-Bass SPMD program (one NeuronCore's view).

    The walrus build in this container allows exactly ONE semaphore wait per
    hardware instruction, which rules out the Tile framework's generated
    sync.  Raw Bass with explicit semaphores is used instead; where an
    instruction needs two producers, two consecutive single-wait
    instructions are emitted.

    Per chunk c:
        PE : po = C_c^T.T @ V_c (+)= A_c^T.T @ S_{c-1}   [128, 64] PSUM
             pn = K_c^T.T @ V_c (+)= P_c^T.T @ S_{c-1}   [ 64, 64] PSUM
             (V-side matmuls issue before the S wait, so only the @S
              matmuls + the DVE state copy sit in the serial chain)
        DVE: S_c   = pn -> ssb    (critical path)
        ACT: Out_c = po -> outsb  (off critical path)
    Input arrives in two half DMAs (chunks 0..15 / 16..31) gated by two
    semaphores so compute starts after the first half; output leaves in two
    half DMAs as soon as each half's copies are done.
    """
    import contextlib

    import concourse.bass as bass
    import concourse.mybir as mybir

    f32 = mybir.dt.float32
    nc = bass.Bass()
    # 64-row regions (A^T, P^T) ship separately so no zero-padding is moved
    inp64_d = nc.declare_dram_parameter("inp64", [D, _ATP_W + _PTP_W], f32,
                                        isOutput=False)
    inp_d = nc.declare_dram_parameter("inp", [B, _CTP_W + _KTP_W + _VAL_W], f32,
                                      isOutput=False)
    out_d = nc.declare_dram_parameter("out", [B, NCK * WS], f32, isOutput=True)

    H = NCK // 2

    with (
        nc.sbuf_tensor([B, INP_W], f32) as inp,
        nc.sbuf_tensor([B, NCK * WS], f32) as outsb,
        nc.sbuf_tensor([D, NCK * WS], f32) as ssb,

        contextlib.ExitStack() as _es,
        nc.Block() as block,
    ):
        dma_qs = [_es.enter_context(nc.semaphore(f"dma_q{i}")) for i in range(GIN)]
        dma_out = _es.enter_context(nc.semaphore("dma_out"))
        pe_sem = _es.enter_context(nc.semaphore("pe_sem"))
        act_sem = _es.enter_context(nc.semaphore("act_sem"))
        dve_sem = _es.enter_context(nc.semaphore("dve_sem"))
        def a_slc(c):
            return inp[0:D, _OFF_ATP + c * B : _OFF_ATP + (c + 1) * B]

        def c_slc(c):
            return inp[:, _OFF_CTP + c * B : _OFF_CTP + (c + 1) * B]

        def p_slc(c):
            return inp[0:D, _OFF_PTP + c * D : _OFF_PTP + (c + 1) * D]

        def k_slc(c):
            return inp[:, _OFF_KTP + c * D : _OFF_KTP + (c + 1) * D]

        def v_slc(c):
            return inp[:, _OFF_VAL + c * WS : _OFF_VAL + (c + 1) * WS]

        pos = [_es.enter_context(nc.psum_tensor(f"po{i}", [B, WS], f32))
               for i in range(NPS)]
        pns = [_es.enter_context(nc.psum_tensor(f"pn{i}", [D, WS], f32))
               for i in range(NPS)]

        Q = NCK // GIN

        @block.tensor
        def _(tensor):
            tensor.wait_ge(dma_qs[0], 16 * 5)
            for c in range(NCK):
                if c % Q == 0 and c > 0:
                    tensor.wait_ge(dma_qs[c // Q], 16 * 5)
                po = pos[c % NPS]
                pn = pns[c % NPS]
                if c == 0:
                    tensor.matmul(pn[:], k_slc(0), v_slc(0),
                                  start=True, stop=True).then_inc(pe_sem, 1)
                    tensor.matmul(po[:], c_slc(0), v_slc(0),
                                  start=True, stop=True).then_inc(pe_sem, 1)
                else:
                    if c >= NPS:
                        # PSUM WAR: chunk c-NPS's consumers must be done
                        tensor.wait_ge(act_sem, c - NPS + 1)
                        tensor.wait_ge(dve_sem, c - NPS + 1)
                    # V-side matmuls: no dependence on the state chain
                    tensor.matmul(po[:], c_slc(c), v_slc(c), start=True, stop=False)
                    tensor.matmul(pn[:], k_slc(c), v_slc(c), start=True, stop=False)
                    # chain-bound: needs S_{c-1}.  P@S (the state production)
                    # goes FIRST so the DVE copy overlaps the A@S matmul.
                    tensor.wait_ge(dve_sem, c)
                    sprev = ssb[:, (c - 1) * WS : c * WS]
                    tensor.matmul(pn[:], p_slc(c), sprev,
                                  start=False, stop=True).then_inc(pe_sem, 1)
                    tensor.matmul(po[:], a_slc(c), sprev,
                                  start=False, stop=True).then_inc(pe_sem, 1)

        @block.vector
        def _(vector):
            for c in range(NCK):
                pn = pns[c % NPS]
                vector.wait_ge(pe_sem, 2 * c + 1)
                vector.tensor_copy(ssb[:, c * WS : (c + 1) * WS],
                                   pn[:]).then_inc(dve_sem, 1)

        @block.scalar
        def _(scalar):
            for c in range(NCK):
                po = pos[c % NPS]
                scalar.wait_ge(pe_sem, 2 * c + 2)
                scalar.copy(outsb[:, c * WS : (c + 1) * WS],
                            po[:]).then_inc(act_sem, 1)

        @block.sync
        def _(sync):
            # (sbuf_off, dram_tensor, dram_off, width, rows)
            regions = (
                (_OFF_ATP, inp64_d, 0, _ATP_W, D),
                (_OFF_PTP, inp64_d, _ATP_W, _PTP_W, D),
                (_OFF_CTP, inp_d, 0, _CTP_W, B),
                (_OFF_KTP, inp_d, _CTP_W, _KTP_W, B),
                (_OFF_VAL, inp_d, _CTP_W + _KTP_W, _VAL_W, B),
            )
            for quarter in range(GIN):
                sem = dma_qs[quarter]
                for soff, dten, doff, wdt, rows in regions:
                    qw = wdt // GIN
                    s0, d0 = soff + quarter * qw, doff + quarter * qw
                    sync.dma_start(
                        out=inp[0:rows, s0 : s0 + qw],
                        in_=dten[:, d0 : d0 + qw]).then_inc(sem, 16)
            QO = NCK // GOUT
            for qtr in range(GOUT):
                sync.wait_ge(act_sem, (qtr + 1) * QO)
                sync.dma_start(
                    out=out_d[:, qtr * QO * WS : (qtr + 1) * QO * WS],
                    in_=outsb[:, qtr * QO * WS : (qtr + 1) * QO * WS],
                ).then_inc(dma_out, 16)
            sync.wait_ge(dma_out, 16 * GOUT)

    return nc


def _make_in_maps(actions, values):
    atp, ctp, ptp, ktp = _build_coeffs(actions)
    inp64 = np.ascontiguousarray(np.concatenate([atp, ptp], axis=1))
    in_maps = []
    for k in range(NCORES):
        vsh = values[:, k * WS : (k + 1) * WS]            # [T, WS]
        vdev = vsh.reshape(NCK, B, WS).transpose(1, 0, 2).reshape(B, NCK * WS)
        inp = np.ascontiguousarray(np.concatenate([ctp, ktp, vdev], axis=1))
        in_maps.append({"inp64": inp64, "inp": inp})
    return in_maps


def kernel(actions, values):
    global _PROG
    from concourse.bass_utils import run_bass_kernel_spmd

    actions = np.asarray(actions, np.float32)
    values = np.asarray(values, np.float32)
    if _PROG is None:
        _PROG = _build_program()
    nc = _PROG
    in_maps = _make_in_maps(actions, values)

    try:
        res = run_bass_kernel_spmd(nc, in_maps, list(range(NCORES)))
    except Exception:
        # one retry: a previously-wedged NeuronCore (e.g. a killed earlier
        # run) surfaces as NRT_EXEC_UNIT_UNRECOVERABLE on first dispatch
        res = run_bass_kernel_spmd(nc, in_maps, list(range(NCORES)))
    global _LAST_RESULT
    _LAST_RESULT = res

    out = np.empty((T, W), np.float32)
    for k in range(NCORES):
        o = res.results[k]["out"]                         # [B, NCK*WS]
        out[:, k * WS : (k + 1) * WS] = (
            o.reshape(B, NCK, WS).transpose(1, 0, 2).reshape(T, WS))
    return out


def sim_time_ns():
    """Cost-model (TimelineSim) estimate of one core's execution time."""
    global _PROG
    if _PROG is None:
        _PROG = _build_program()
    from concourse.timeline_sim import TimelineSim

    return float(TimelineSim(_PROG, no_exec=True).simulate())


# revision 49
# speedup vs baseline: 1.0054x; 1.0054x over previous
"""Neural-stack kernel for Trainium2 (8 NeuronCores, SPMD, W-sharded).

Math: the reference recurrence
    stack_t = (noop+pop)_t * stack_{t-1} + push_t * shift_down(stack_{t-1}, v_t)
is linear in the stack given per-step scalars.  All step matrices are
polynomials in the down-shift Z (nilpotent, Z^64 = 0), so products over a
chunk of B steps stay lower-triangular-Toeplitz.  Per chunk c (B=128 steps)
the outputs and the chunk-boundary state obey
    Out_c   = A_c @ S_c + C_c @ V_c          (B x W)
    S_{c+1} = P_c @ S_c + K_c @ V_c          (D x W)
with small coefficient matrices A/C/P/K derived from the (tiny, strictly
sequential, nonlinear) strength recurrence.  The host computes the strength
recurrence + coefficients from `actions` (48KB of input); the 8 cores each
run the chunked matmul recurrence over their 64-column shard of `values`
(99.7% of the FLOPs and bytes).
"""

import sys
import numpy as np

for _p in ("/opt/trn_rl_repo", "/root/.axon_site/_ro/trn_rl_repo"):
    if _p not in sys.path:
        sys.path.append(_p)

T, D, W = 4096, 64, 512
B = 128               # chunk length (time steps per chunk)
NCK = T // B          # 32 chunks
NCORES = 8
WS = W // NCORES      # 64 columns per core
EPS = np.float32(1e-6)


# ---------------------------------------------------------------- host side
def _build_coeffs(actions):
    from numpy.lib.stride_tricks import sliding_window_view

    err = np.seterr(all="ignore")
    a = np.asarray(actions, np.float32)

    # strength recurrence, step-exact f32 (matches the reference's fp32 path)
    s = np.zeros(D, np.float32)
    sig = np.zeros((T + 1, D), np.float32)
    g = np.zeros(T + 1, np.float32)
    for t in range(1, T + 1):
        push, pop, noop = a[t - 1]
        sig[t] = s
        tot = s.sum(dtype=np.float32)
        g[t] = (np.float32(1) / (tot + EPS)) if tot > EPS else np.float32(0)
        cum_above = (np.cumsum(s[::-1], dtype=np.float32)[::-1] - s).astype(np.float32)
        red = np.minimum(np.maximum(pop - cum_above, np.float32(0)), s)
        s = noop * s + pop * (s - red) + push * np.concatenate([[push], s[:-1]]).astype(np.float32)
        tot2 = s.sum(dtype=np.float32)
        if tot2 > D:
            s *= np.float32(D) / tot2

    sv = (a[:, 2] + a[:, 1]).reshape(NCK, B)
    pv = a[:, 0].reshape(NCK, B).copy()
    sigc = sig[1:].reshape(NCK, B, D)
    gc = g[1:].reshape(NCK, B)

    # prefix coefficient vectors rho[c, i] of prod_{tau<=i} (s + p Z) mod Z^D
    rho = np.zeros((NCK, B + 1, D), np.float32)
    rho[:, 0, 0] = 1
    for i in range(1, B + 1):
        st = sv[:, i - 1 : i]
        pt = pv[:, i - 1 : i]
        r = rho[:, i - 1]
        rho[:, i, 0] = st[:, 0] * r[:, 0]
        rho[:, i, 1:] = st * r[:, 1:] + pt * r[:, :-1]

    # A[c, i, k] = g * sum_m sig[c,i,k+m] * rho[c,i,m]
    sig_pad = np.concatenate([sigc, np.zeros((NCK, B, D), np.float32)], axis=2)
    win = sliding_window_view(sig_pad, D, axis=2)
    A = np.einsum("cikm,cim->cik", win[:, :, :D, :].astype(np.float32),
                  rho[:, :B], optimize=True).astype(np.float32)
    A *= gc[:, :, None]

    # interval products q^{(j)} = coeffs of prod_{tau=j+1}^{i-1} (s+pZ) mod Z^D
    # (one slot per start j, evolved in lockstep) -> C and K
    q = np.zeros((NCK, B, D), np.float32)
    q[:, :, 0] = 1.0
    C = np.zeros((NCK, B, B), np.float32)
    for i in range(2, B + 1):
        if i >= 3:
            st = sv[:, i - 2 : i - 1]
            pt = pv[:, i - 2 : i - 1]
            qs = q[:, : i - 2]
            q[:, : i - 2, 1:] = st[:, :, None] * qs[:, :, 1:] + pt[:, :, None] * qs[:, :, :-1]
            q[:, : i - 2, 0] = st * qs[:, :, 0]
        dots = np.einsum("cm,cjm->cj", sigc[:, i - 1], q[:, : i - 1],
                         optimize=True).astype(np.float32)
        C[:, i - 1, : i - 1] = gc[:, i - 1 : i] * pv[:, : i - 1] * dots
    # bring every q^{(j)} up to the full-chunk product prod_{j+1}^{B}
    st = sv[:, B - 1 : B]
    pt = pv[:, B - 1 : B]
    qs = q[:, : B - 1]
    q[:, : B - 1, 1:] = st[:, :, None] * qs[:, :, 1:] + pt[:, :, None] * qs[:, :, :-1]
    q[:, : B - 1, 0] = st * qs[:, :, 0]
    K = (pv[:, None, :] * q.transpose(0, 2, 1)).astype(np.float32)   # [c, D, B]

    P = np.zeros((NCK, D, D), np.float32)
    rB = rho[:, B]
    for k in range(D):
        P[:, k:, k] = rB[:, : D - k]

    np.seterr(**err)
    # device layouts: chunk-stacked transposed (lhsT) matrices
    atp = np.ascontiguousarray(A.transpose(2, 0, 1).reshape(D, NCK * B))    # [64, 4096]
    ctp = np.ascontiguousarray(C.transpose(2, 0, 1).reshape(B, NCK * B))    # [128, 4096]
    ptp = np.ascontiguousarray(P.transpose(2, 0, 1).reshape(D, NCK * D))    # [64, 2048]
    ktp = np.ascontiguousarray(K.transpose(2, 0, 1).reshape(B, NCK * D))    # [128, 2048]
    return atp, ctp, ptp, ktp


# ---------------------------------------------------------------- device side
_PROG = None

# tuning knobs (sweepable): input DMA groups, output DMA groups, PSUM depth
GIN = 4
GOUT = 16
NPS = 2

# input region column offsets inside the [128, INP_W] device tensor
_ATP_W, _CTP_W, _PTP_W, _KTP_W, _VAL_W = NCK * B, NCK * B, NCK * D, NCK * D, NCK * WS
_OFF_ATP = 0
_OFF_CTP = _OFF_ATP + _ATP_W
_OFF_PTP = _OFF_CTP + _CTP_W
_OFF_KTP = _OFF_PTP + _PTP_W
_OFF_VAL = _OFF_KTP + _KTP_W
INP_W = _OFF_VAL + _VAL_W


def _build_program():
    """Raw-Bass SPMD program (one NeuronCore's view).

    The walrus build in this container allows exactly ONE semaphore wait per
    hardware instruction, which rules out the Tile framework's generated
    sync.  Raw Bass with explicit semaphores is used instead; where an
    instruction needs two producers, two consecutive single-wait
    instructions are emitted.

    Per chunk c:
        PE : po = C_c^T.T @ V_c (+)= A_c^T.T @ S_{c-1}   [128, 64] PSUM
             pn = K_c^T.T @ V_c (+)= P_c^T.T @ S_{c-1}   [ 64, 64] PSUM
             (V-side matmuls issue before the S wait, so only the @S
              matmuls + the DVE state copy sit in the serial chain)
        DVE: S_c   = pn -> ssb    (critical path)
        ACT: Out_c = po -> outsb  (off critical path)
    Input arrives in two half DMAs (chunks 0..15 / 16..31) gated by two
    semaphores so compute starts after the first half; output leaves in two
    half DMAs as soon as each half's copies are done.
    """
    import contextlib

    import concourse.bass as bass
    import concourse.mybir as mybir

    f32 = mybir.dt.float32
    nc = bass.Bass()
    # 64-row regions (A^T, P^T) ship separately so no zero-padding is moved
    inp64_d = nc.declare_dram_parameter("inp64", [D, _ATP_W + _PTP_W], f32,
                                        isOutput=False)
    inp_d = nc.declare_dram_parameter("inp", [B, _CTP_W + _KTP_W + _VAL_W], f32,
                                      isOutput=False)
    out_d = nc.declare_dram_parameter("out", [B, NCK * WS], f32, isOutput=True)

    H = NCK // 2

    with (
        nc.sbuf_tensor([B, INP_W], f32) as inp,
        nc.sbuf_tensor([B, NCK * WS], f32) as outsb,
        nc.sbuf_tensor([D, NCK * WS], f32) as ssb,

        contextlib.ExitStack() as _es,
        nc.Block() as block,
    ):
        dma_qs = [_es.enter_context(nc.semaphore(f"dma_q{i}")) for i in range(GIN)]
        dma_out = _es.enter_context(nc.semaphore("dma_out"))
        pe_sem = _es.enter_context(nc.semaphore("pe_sem"))
        act_sem = _es.enter_context(nc.semaphore("act_sem"))
        dve_sem = _es.enter_context(nc.semaphore("dve_sem"))
        def a_slc(c):
            return inp[0:D, _OFF_ATP + c * B : _OFF_ATP + (c + 1) * B]

        def c_slc(c):
            return inp[:, _OFF_CTP + c * B : _OFF_CTP + (c + 1) * B]

        def p_slc(c):
            return inp[0:D, _OFF_PTP + c * D : _OFF_PTP + (c + 1) * D]

        def k_slc(c):
            return inp[:, _OFF_KTP + c * D : _OFF_KTP + (c + 1) * D]

        def v_slc(c):
            return inp[:, _OFF_VAL + c * WS : _OFF_VAL + (c + 1) * WS]

        pos = [_es.enter_context(nc.psum_tensor(f"po{i}", [B, WS], f32))
               for i in range(NPS)]
        pns = [_es.enter_context(nc.psum_tensor(f"pn{i}", [D, WS], f32))
               for i in range(NPS)]

        Q = NCK // GIN

        @block.tensor
        def _(tensor):
            tensor.wait_ge(dma_qs[0], 16 * 5)
            for c in range(NCK):
                if c % Q == 0 and c > 0:
                    tensor.wait_ge(dma_qs[c // Q], 16 * 5)
                po = pos[c % NPS]
                pn = pns[c % NPS]
                if c == 0:
                    tensor.matmul(pn[:], k_slc(0), v_slc(0),
                                  start=True, stop=True).then_inc(pe_sem, 1)
                    tensor.matmul(po[:], c_slc(0), v_slc(0),
                                  start=True, stop=True).then_inc(pe_sem, 1)
                else:
                    if c >= NPS:
                        # PSUM WAR: chunk c-NPS's consumers must be done
                        tensor.wait_ge(act_sem, c - NPS + 1)
                        tensor.wait_ge(dve_sem, c - NPS + 1)
                    # V-side matmuls: no dependence on the state chain
                    tensor.matmul(po[:], c_slc(c), v_slc(c), start=True, stop=False)
                    tensor.matmul(pn[:], k_slc(c), v_slc(c), start=True, stop=False)
                    # chain-bound: needs S_{c-1}.  P@S (the state production)
                    # goes FIRST so the DVE copy overlaps the A@S matmul.
                    tensor.wait_ge(dve_sem, c)
                    sprev = ssb[:, (c - 1) * WS : c * WS]
                    tensor.matmul(pn[:], p_slc(c), sprev,
                                  start=False, stop=True).then_inc(pe_sem, 1)
                    tensor.matmul(po[:], a_slc(c), sprev,
                                  start=False, stop=True).then_inc(pe_sem, 1)

        @block.vector
        def _(vector):
            for c in range(NCK):
                pn = pns[c % NPS]
                vector.wait_ge(pe_sem, 2 * c + 1)
                vector.tensor_copy(ssb[:, c * WS : (c + 1) * WS],
                                   pn[:]).then_inc(dve_sem, 1)

        @block.scalar
        def _(scalar):
            for c in range(NCK):
                po = pos[c % NPS]
                scalar.wait_ge(pe_sem, 2 * c + 2)
                scalar.copy(outsb[:, c * WS : (c + 1) * WS],
                            po[:]).then_inc(act_sem, 1)

        @block.sync
        def _(sync):
            # (sbuf_off, dram_tensor, dram_off, width, rows)
            regions = (
                (_OFF_ATP, inp64_d, 0, _ATP_W, D),
                (_OFF_PTP, inp64_d, _ATP_W, _PTP_W, D),
                (_OFF_CTP, inp_d, 0, _CTP_W, B),
                (_OFF_KTP, inp_d, _CTP_W, _KTP_W, B),
                (_OFF_VAL, inp_d, _CTP_W + _KTP_W, _VAL_W, B),
            )
            for quarter in range(GIN):
                sem = dma_qs[quarter]
                for soff, dten, doff, wdt, rows in regions:
                    qw = wdt // GIN
                    s0, d0 = soff + quarter * qw, doff + quarter * qw
                    sync.dma_start(
                        out=inp[0:rows, s0 : s0 + qw],
                        in_=dten[:, d0 : d0 + qw]).then_inc(sem, 16)
            QO = NCK // GOUT
            for qtr in range(GOUT):
                sync.wait_ge(act_sem, (qtr + 1) * QO)
                sync.dma_start(
                    out=out_d[:, qtr * QO * WS : (qtr + 1) * QO * WS],
                    in_=outsb[:, qtr * QO * WS : (qtr + 1) * QO * WS],
                ).then_inc(dma_out, 16)
            sync.wait_ge(dma_out, 16 * GOUT)

    return nc


def _make_in_maps(actions, values):
    atp, ctp, ptp, ktp = _build_coeffs(actions)
    inp64 = np.ascontiguousarray(np.concatenate([atp, ptp], axis=1))
    in_maps = []
    for k in range(NCORES):
        vsh = values[:, k * WS : (k + 1) * WS]            # [T, WS]
        vdev = vsh.reshape(NCK, B, WS).transpose(1, 0, 2).reshape(B, NCK * WS)
        inp = np.ascontiguousarray(np.concatenate([ctp, ktp, vdev], axis=1))
        in_maps.append({"inp64": inp64, "inp": inp})
    return in_maps


def kernel(actions, values):
    global _PROG
    from concourse.bass_utils import run_bass_kernel_spmd

    actions = np.asarray(actions, np.float32)
    values = np.asarray(values, np.float32)
    if _PROG is None:
        _PROG = _build_program()
    nc = _PROG
    in_maps = _make_in_maps(actions, values)

    try:
        res = run_bass_kernel_spmd(nc, in_maps, list(range(NCORES)))
    except Exception:
        # one retry: a previously-wedged NeuronCore (e.g. a killed earlier
        # run) surfaces as NRT_EXEC_UNIT_UNRECOVERABLE on first dispatch
        res = run_bass_kernel_spmd(nc, in_maps, list(range(NCORES)))
    global _LAST_RESULT
    _LAST_RESULT = res

    out = np.empty((T, W), np.float32)
    for k in range(NCORES):
        o = res.results[k]["out"]                         # [B, NCK*WS]
        out[:, k * WS : (k + 1) * WS] = (
            o.reshape(B, NCK, WS).transpose(1, 0, 2).reshape(T, WS))
    return out


def sim_time_ns():
    """Cost-model (TimelineSim) estimate of one core's execution time."""
    global _PROG
    if _PROG is None:
        _PROG = _build_program()
    from concourse.timeline_sim import TimelineSim

    return float(TimelineSim(_PROG, no_exec=True).simulate())
